# revision 4
# baseline (speedup 1.0000x reference)
"""AnchorGenerator Bass kernel for 8 Trainium2 NeuronCores.

Generates multi-level anchor boxes: for each of 4 feature-map levels
(stride 4/8/16/32, sizes 1024^2/512^2/256^2/128^2), the output is
[9*h*w, 4] f32 rows (cx, cy, aw, ah), ordered (anchor_type, y, x).

Strategy (memory-bound, pure output generation ~200MB):
  - Shard each level's y (row) dimension across the 8 cores: core k owns
    y in [k*h/8, (k+1)*h/8) for all 9 anchor types.
  - Per core+level, the shard is [9*hc*w, 4] = [9*hc row-groups, 4w floats]
    laid out contiguously; tiles of 128 row-groups are built in SBUF and
    DMA'd out as single large contiguous transfers.
  - Tile content: col0 = xc (iota-generated, same every row-group),
    col1/2/3 = per-row-group constants (yc / anchor-w / anchor-h) written
    via per-partition-scalar ops from tiny host-precomputed tables.
"""

import math

import numpy as np

import concourse.bass as bass
import concourse.bacc as bacc
import concourse.mybir as mybir
from concourse.tile import TileContext
from concourse.bass_utils import run_bass_kernel_spmd

NCORES = 8
STRIDES = [4, 8, 16, 32]
SIZES = [(1024, 1024), (512, 512), (256, 256), (128, 128)]
S = 3  # scales
R = 3  # ratios
A = S * R

# Per-level derived constants: (h, w, stride, hc, G, T)
#   hc = rows of y per core, G = row-groups per core, T = 128-row tiles
LEVELS = []
for (_h, _w), _s in zip(SIZES, STRIDES):
    _hc = _h // NCORES
    _G = A * _hc
    LEVELS.append((_h, _w, _s, _hc, _G, (_G + 127) // 128))
TTOT = sum(lv[5] for lv in LEVELS)  # total tiles per core

_F32 = mybir.dt.float32

_BUILT = None  # cached (nc,) build


def _build():
    """Build the Bass program (identical on all 8 cores; per-core data
    arrives via the small scalar-table inputs)."""
    nc = bacc.Bacc()

    ycol = nc.dram_tensor("ycol", [128, TTOT], _F32, kind="ExternalInput")
    awcol = nc.dram_tensor("awcol", [128, TTOT], _F32, kind="ExternalInput")
    ahcol = nc.dram_tensor("ahcol", [128, TTOT], _F32, kind="ExternalInput")
    outs = [
        nc.dram_tensor(f"out{L}", [G, 4 * w], _F32, kind="ExternalOutput")
        for L, (h, w, s, hc, G, T) in enumerate(LEVELS)
    ]

    ident = mybir.ActivationFunctionType.Identity
    with TileContext(nc) as tc:
        with (
            tc.tile_pool(name="consts", bufs=1) as cpool,
            tc.tile_pool(name="work", bufs=8) as wpool,
        ):
            yc_sb = cpool.tile([128, TTOT], _F32, tag="yc")
            aw_sb = cpool.tile([128, TTOT], _F32, tag="aw")
            ah_sb = cpool.tile([128, TTOT], _F32, tag="ah")
            nc.sync.dma_start(out=yc_sb[:, :], in_=ycol[:, :])
            nc.sync.dma_start(out=aw_sb[:, :], in_=awcol[:, :])
            nc.sync.dma_start(out=ah_sb[:, :], in_=ahcol[:, :])

            tbase = 0
            for L, (h, w, stride, hc, G, T) in enumerate(LEVELS):
                # xc[p, x] = x*stride + stride//2 (exact small ints in f32)
                xc = cpool.tile([128, w], _F32, tag=f"xc{L}")
                nc.gpsimd.iota(
                    xc[:, :],
                    pattern=[[stride, w]],
                    base=stride // 2,
                    channel_multiplier=0,
                    allow_small_or_imprecise_dtypes=True,
                )
                for t in range(T):
                    P = min(128, G - t * 128)
                    tile = wpool.tile([128, 4 * w], _F32, tag="buf")
                    tv = tile[:P, :].rearrange("p (x c) -> p x c", c=4)
                    tcol = tbase + t
                    # col 0: x centers (varying along free dim)
                    nc.vector.tensor_copy(tv[:, :, 0], xc[:P, :])
                    # col 1: y center, constant per row-group
                    nc.scalar.activation(
                        tv[:, :, 1], xc[:P, :], ident,
                        bias=yc_sb[:P, tcol : tcol + 1], scale=0.0,
                    )
                    # col 2: anchor width, constant per row-group
                    nc.vector.tensor_scalar(
                        tv[:, :, 2], xc[:P, :],
                        0.0, aw_sb[:P, tcol : tcol + 1],
                        mybir.AluOpType.mult, mybir.AluOpType.add,
                    )
                    # col 3: anchor height, constant per row-group
                    nc.scalar.activation(
                        tv[:, :, 3], xc[:P, :], ident,
                        bias=ah_sb[:P, tcol : tcol + 1], scale=0.0,
                    )
                    nc.sync.dma_start(
                        out=outs[L][t * 128 : t * 128 + P, :], in_=tile[:P, :]
                    )
                tbase += T
    nc.finalize()
    return nc


def _tables(scales, ratios):
    """Per-core per-partition scalar tables [8, 128, TTOT] f32 for
    (y center, anchor width, anchor height), one column per tile."""
    scales = np.asarray(scales, dtype=np.float32)
    ratios = np.asarray(ratios, dtype=np.float32)
    ycols = np.zeros((NCORES, 128, TTOT), np.float32)
    awcols = np.zeros((NCORES, 128, TTOT), np.float32)
    ahcols = np.zeros((NCORES, 128, TTOT), np.float32)
    tbase = 0
    for (h, w, stride, hc, G, T) in LEVELS:
        base = scales * np.float32(stride)          # [S] f32
        sr = np.sqrt(ratios)                        # [R] f32
        aw = (base[:, None] * sr[None, :]).reshape(-1)  # [A] f32
        ah = (base[:, None] / sr[None, :]).reshape(-1)  # [A] f32
        for t in range(T):
            g = t * 128 + np.arange(128)
            gg = np.minimum(g, G - 1)  # pad region: harmless values
            a = gg // hc
            yloc = gg % hc
            for k in range(NCORES):
                yglob = k * hc + yloc
                ycols[k, :, tbase + t] = (yglob * stride + stride // 2).astype(
                    np.float32
                )
                awcols[k, :, tbase + t] = aw[a]
                ahcols[k, :, tbase + t] = ah[a]
        tbase += T
    return ycols, awcols, ahcols


def _get_built():
    global _BUILT
    if _BUILT is None:
        _BUILT = _build()
    return _BUILT


def _run(scales, ratios, **spmd_kwargs):
    nc = _get_built()
    ycols, awcols, ahcols = _tables(scales, ratios)
    in_maps = [
        {"ycol": ycols[k], "awcol": awcols[k], "ahcol": ahcols[k]}
        for k in range(NCORES)
    ]
    res = run_bass_kernel_spmd(nc, in_maps, core_ids=list(range(NCORES)),
                               **spmd_kwargs)
    outs = []
    for L, (h, w, stride, hc, G, T) in enumerate(LEVELS):
        full = np.empty((A, h, 4 * w), np.float32)
        for k in range(NCORES):
            full[:, k * hc : (k + 1) * hc, :] = res.results[k][f"out{L}"].reshape(
                A, hc, 4 * w
            )
        outs.append(full.reshape(A * h * w, 4))
    return tuple(outs), res


def kernel(scales, ratios, fs0_h, fs0_w, fs1_h, fs1_w, fs2_h, fs2_w,
           fs3_h, fs3_w):
    sizes = [(int(fs0_h), int(fs0_w)), (int(fs1_h), int(fs1_w)),
             (int(fs2_h), int(fs2_w)), (int(fs3_h), int(fs3_w))]
    assert sizes == SIZES, f"kernel compiled for {SIZES}, got {sizes}"
    outs, _ = _run(scales, ratios)
    return outs
